# revision 2
# baseline (speedup 1.0000x reference)
"""Gridnet Trainium2 kernel — self-contained.

Blockwise-normalized 27-neighbor gridnet step (8 inner iterations,
block_size 8) for x:[4,128,128,128] f32 with per-cell weights
w:[27,128,128,128], bias, residual_scale.

Strategy
--------
Shard the M axis across the 8 NeuronCores (16 M-cells = 2 blocks of 8 per
core). Blocks carry a frozen 1-cell halo from the initial activations, so
cores never communicate.

Per core: 512 spatial blocks x 4 batches. On-chip layout: 4 tiles of 128
blocks (partition = block), free dim = padded cells x batch stored as
[i(10), j(10), k(10), b(4)] so any stencil window merges (k,b) into one
contiguous stride-1 dim of 32 -> all stencil passes hit the DVE bf16 2x
mode: window AP [(i:400,8),(j:40,8),(kb:1,32)].

Math: since normalization is affine per block, run the 27-offset weighted
sum S on RAW activations and correct afterward:
    acc = istd*(S - mu*sw + std*bias),  sw = sum_o w_o (precomputed)
so `normed` is never materialized. Stats (sum/sumsq) run on the Scalar
engine (activation accum_out), stencil products/adds on the Vector engine
in bf16, silu on ScalarE, residual update in place.

Weights are streamed from HBM each iteration (too big for SBUF),
pre-replicated x4 over batch to match the (k,b)-merged layout.
"""
import numpy as np
import ml_dtypes

BS = 8
EPS = 1e-5
NB = 16           # blocks per axis globally
N_CORES = 8
BF = ml_dtypes.bfloat16

_CACHE = {}


# ---------------------------------------------------------------- host prep

def _blk(v):
    """[..., 16,128,128] -> [..., 4(tile),128(p),512(cell)] for one core's M-slab."""
    L = v.shape[:-3]
    vr = v.reshape(*L, 2, 8, 2, 8, 8, 16, 8)   # g, ii, nh, nl, jj, kb, kk
    n = len(L)
    perm = tuple(range(n)) + tuple(n + i for i in (0, 2, 3, 5, 1, 4, 6))
    return vr.transpose(perm).reshape(*L, 4, 128, 512)


def _prep_inputs(weight, bias, residual_scale, x):
    xp = np.pad(x, ((0, 0), (1, 1), (1, 1), (1, 1)))
    swv = np.lib.stride_tricks.sliding_window_view(xp, (10, 10, 10), axis=(1, 2, 3))
    blocks = swv[:, ::8, ::8, ::8]            # [4b,16,16,16,10,10,10]
    wsum = weight.sum(axis=0)                 # [128,128,128] f32

    in_maps = []
    for c in range(N_CORES):
        g2 = blocks[:, 2 * c:2 * c + 2]       # [4b,2g,16n,16kb,10,10,10]
        arr = g2.reshape(4, 2, 2, 8, 16, 10, 10, 10)
        arr = arr.transpose(1, 2, 3, 4, 5, 6, 7, 0)   # g,nh,nl,kb,ci,cj,ck,b
        a_host = np.ascontiguousarray(arr).reshape(4, 128, 4000).astype(BF)

        wb = _blk(weight[:, 16 * c:16 * c + 16])      # [27,4,128,512]
        w4 = np.repeat(wb.transpose(1, 0, 2, 3).astype(BF)[..., None], 4, axis=-1)
        w4 = w4.reshape(4, 9, 3, 128, 2048).transpose(0, 1, 3, 2, 4)
        w4 = np.ascontiguousarray(w4).reshape(4, 9, 128, 6144)

        sw_b = np.ascontiguousarray(_blk(wsum[16 * c:16 * c + 16]))          # f32
        bias_b = np.ascontiguousarray(_blk(bias[16 * c:16 * c + 16]).astype(BF))
        rs_b = _blk(residual_scale[16 * c:16 * c + 16]).astype(BF)
        rs4 = np.ascontiguousarray(np.repeat(rs_b[..., None], 4, axis=-1)
                                   ).reshape(4, 128, 2048)
        in_maps.append({'a_in': a_host, 'w4_in': w4, 'sw_in': sw_b,
                        'bias_in': bias_b, 'rs4_in': rs4})
    return in_maps


def _unpack_outputs(results):
    y = np.empty((4, 128, 128, 128), np.float32)
    for c in range(N_CORES):
        out = results[c]['out_f']             # [4,128,2048] f32
        arr = out.reshape(2, 2, 8, 16, 8, 8, 8, 4)  # g,nh,nl,kb,ii,jj,kk,b
        yt = arr.transpose(7, 0, 4, 1, 2, 5, 3, 6)  # b,g,ii,nh,nl,jj,kb,kk
        y[:, 16 * c:16 * c + 16] = yt.reshape(4, 16, 128, 128)
    return y


# ---------------------------------------------------------------- device code

def _build(iters):
    import concourse.bass as bass
    import concourse.tile as tile
    from concourse import bacc, mybir

    BF16 = mybir.dt.bfloat16
    F32 = mybir.dt.float32
    ALU = mybir.AluOpType
    ACTF = mybir.ActivationFunctionType

    def wap(t, offset, dims):
        ap = t.ap().copy()
        ap.ap = mybir.VecI64Pair([list(t.ap().ap[0])] + [list(d) for d in dims])
        ap.offset = offset
        return ap

    nc = bacc.Bacc('TRN2', target_bir_lowering=False, debug=False)
    a_in = nc.dram_tensor('a_in', [4, 128, 4000], BF16, kind='ExternalInput').ap()
    w4_in = nc.dram_tensor('w4_in', [4, 9, 128, 6144], BF16, kind='ExternalInput').ap()
    sw_in = nc.dram_tensor('sw_in', [4, 128, 512], F32, kind='ExternalInput').ap()
    bias_in = nc.dram_tensor('bias_in', [4, 128, 512], BF16, kind='ExternalInput').ap()
    rs4_in = nc.dram_tensor('rs4_in', [4, 128, 2048], BF16, kind='ExternalInput').ap()
    out_f = nc.dram_tensor('out_f', [4, 128, 2048], F32, kind='ExternalOutput').ap()

    INTER = [[400, 8], [40, 8], [1, 32]]      # interior window dims

    with tile.TileContext(nc) as tc:
        with (
            tc.tile_pool(name='persist', bufs=1) as ppool,
            tc.tile_pool(name='wstream', bufs=4) as wpool,
            tc.tile_pool(name='work', bufs=2) as kpool,
            tc.tile_pool(name='small', bufs=2) as spool,
        ):
            A = []
            for t in range(4):
                at = ppool.tile([128, 4000], BF16, tag=f'A{t}')
                nc.sync.dma_start(at[:], a_in[t])
                A.append(at)
            swt, biast, rst = [], [], []
            for t in range(4):
                s = ppool.tile([128, 512], F32, tag=f'sw{t}')
                nc.sync.dma_start(s[:], sw_in[t])
                swt.append(s)
                bb = ppool.tile([128, 512], BF16, tag=f'bias{t}')
                nc.sync.dma_start(bb[:], bias_in[t])
                biast.append(bb)
                r = ppool.tile([128, 2048], BF16, tag=f'rs{t}')
                nc.sync.dma_start(r[:], rs4_in[t])
                rst.append(r)

            for it in range(iters):
                last = it == iters - 1
                for t in range(4):
                    At = A[t]
                    # ---- stats on ScalarE: per-batch sum & sumsq over 1000 cells
                    s_t = spool.tile([128, 4], F32, tag='s')
                    q_t = spool.tile([128, 4], F32, tag='q')
                    junk = spool.tile([128, 1000], BF16, tag='junk')
                    junkf = spool.tile([128, 1000], F32, tag='junkf')
                    for b in range(4):
                        cells = wap(At.tensor, b, [[4, 1000]])
                        nc.scalar.activation(junk[:], cells, ACTF.Copy,
                                             accum_out=s_t[:, b:b + 1])
                        nc.scalar.activation(junkf[:], cells, ACTF.Square,
                                             accum_out=q_t[:, b:b + 1])
                    # ---- stat scalars: negmu, std, istd  [128,4]
                    negmu = spool.tile([128, 4], F32, tag='negmu')
                    nc.vector.tensor_scalar_mul(negmu[:], s_t[:], -1.0 / 1000.0)
                    m2 = spool.tile([128, 4], F32, tag='m2')
                    nc.vector.tensor_scalar_mul(m2[:], q_t[:], 1.0 / 1000.0)
                    musq = spool.tile([128, 4], F32, tag='musq')
                    nc.vector.tensor_tensor(musq[:], negmu[:], negmu[:], ALU.mult)
                    var = spool.tile([128, 4], F32, tag='var')
                    nc.vector.tensor_tensor(var[:], m2[:], musq[:], ALU.subtract)
                    nc.vector.tensor_scalar_add(var[:], var[:], EPS)
                    std = spool.tile([128, 4], F32, tag='std')
                    nc.scalar.activation(std[:], var[:], ACTF.Sqrt)
                    istd = spool.tile([128, 4], F32, tag='istd')
                    nc.vector.reciprocal(istd[:], std[:])

                    # ---- stencil: acc = sum_o w4_o * window(A, o)   (bf16, 2x)
                    acc = kpool.tile([128, 2048], BF16, tag='acc')
                    for c9 in range(9):
                        wt = wpool.tile([128, 6144], BF16, tag='w')
                        nc.sync.dma_start(wt[:], w4_in[t, c9])
                        for ol in range(3):
                            o = c9 * 3 + ol
                            di, dj, dk = o // 9, (o // 3) % 3, o % 3
                            win = wap(At.tensor, di * 400 + dj * 40 + dk * 4, INTER)
                            wsl = wt[:, ol * 2048:(ol + 1) * 2048]
                            if o == 0:
                                nc.vector.tensor_tensor(acc[:], wsl, win, ALU.mult)
                            else:
                                tmp = kpool.tile([128, 2048], BF16, tag='tmp')
                                nc.vector.tensor_tensor(tmp[:], wsl, win, ALU.mult)
                                nc.vector.tensor_tensor(acc[:], acc[:], tmp[:], ALU.add)

                    # ---- correction + silu:
                    # U = S - mu*sw ; V = U + std*bias ; silu_out = Silu(istd*V)
                    U = kpool.tile([128, 2048], F32, tag='U')
                    V = kpool.tile([128, 2048], F32, tag='V')
                    sil = kpool.tile([128, 2048], BF16, tag='sil')
                    for b in range(4):
                        accb = wap(acc.tensor, b, [[4, 512]])
                        Ub = wap(U.tensor, b, [[4, 512]])
                        Vb = wap(V.tensor, b, [[4, 512]])
                        silb = wap(sil.tensor, b, [[4, 512]])
                        nc.vector.scalar_tensor_tensor(
                            Ub, swt[t][:], negmu[:, b:b + 1], accb, ALU.mult, ALU.add)
                        nc.vector.scalar_tensor_tensor(
                            Vb, biast[t][:], std[:, b:b + 1], Ub, ALU.mult, ALU.add)
                        nc.scalar.activation(silb, Vb, ACTF.Silu,
                                             scale=istd[:, b:b + 1])
                    # ---- residual update
                    delta = kpool.tile([128, 2048], BF16, tag='delta')
                    nc.vector.tensor_tensor(delta[:], rst[t][:], sil[:], ALU.mult)
                    awin = wap(At.tensor, 444, INTER)
                    if not last:
                        nc.vector.tensor_tensor(awin, awin, delta[:], ALU.add)
                    else:
                        fo = kpool.tile([128, 2048], F32, tag='fo')
                        nc.vector.tensor_tensor(fo[:], awin, delta[:], ALU.add)
                        nc.sync.dma_start(out_f[t], fo[:])
    nc.compile()
    return nc


# ---------------------------------------------------------------- entry point

def _run_hw(weight, bias, residual_scale, x, iters):
    from concourse.bass_utils import run_bass_kernel_spmd
    key = iters
    if key not in _CACHE:
        _CACHE[key] = _build(iters)
    nc = _CACHE[key]
    in_maps = _prep_inputs(weight, bias, residual_scale, x)
    res = run_bass_kernel_spmd(nc, in_maps, core_ids=list(range(N_CORES)))
    return _unpack_outputs(res.results)


def _run_numpy(weight, bias, residual_scale, x, it):
    # reference fallback (pure numpy), used only if the HW path fails
    w = _np_blockify_param(weight)
    b = _np_blockify_param(bias)
    rs = _np_blockify_param(residual_scale)
    xp = np.pad(x, ((0, 0), (1, 1), (1, 1), (1, 1)))
    sw = np.lib.stride_tricks.sliding_window_view(xp, (10, 10, 10), axis=(1, 2, 3))
    acts = np.ascontiguousarray(sw[:, ::8, ::8, ::8])
    inter = slice(1, 9)
    for _ in range(it):
        mu = acts.mean(axis=(-3, -2, -1), keepdims=True)
        var = acts.var(axis=(-3, -2, -1), keepdims=True)
        normed = (acts - mu) / np.sqrt(var + EPS)
        acc = np.broadcast_to(b[None], (4,) + b.shape).copy()
        o = 0
        for di in range(3):
            for dj in range(3):
                for dk in range(3):
                    acc += w[o][None] * normed[..., di:di + 8, dj:dj + 8, dk:dk + 8]
                    o += 1
        acts[..., inter, inter, inter] += rs[None] * (acc / (1.0 + np.exp(-acc)))
    out = acts[..., inter, inter, inter]
    return out.transpose(0, 1, 4, 2, 5, 3, 6).reshape(4, 128, 128, 128).astype(np.float32)


def _np_blockify_param(p):
    lead = p.shape[:-3]
    y = p.reshape(*lead, 16, 8, 16, 8, 16, 8)
    n = len(lead)
    perm = tuple(range(n)) + (n, n + 2, n + 4, n + 1, n + 3, n + 5)
    return np.transpose(y, perm)


def kernel(weight, bias, residual_scale, x, inner_iterations, block_size):
    weight = np.asarray(weight, np.float32)
    bias = np.asarray(bias, np.float32)
    residual_scale = np.asarray(residual_scale, np.float32)
    x = np.asarray(x, np.float32)
    it = int(inner_iterations)
    assert int(block_size) == BS and x.shape == (4, 128, 128, 128)
    try:
        return _run_hw(weight, bias, residual_scale, x, it)
    except Exception:
        import traceback
        traceback.print_exc()
        return _run_numpy(weight, bias, residual_scale, x, it)


# revision 6
# speedup vs baseline: 1.2871x; 1.2871x over previous
"""Gridnet Trainium2 kernel — self-contained.

Blockwise-normalized 27-neighbor gridnet step (8 inner iterations,
block_size 8) for x:[4,128,128,128] f32 with per-cell weights
w:[27,128,128,128], bias, residual_scale.

Strategy
--------
Shard the M axis across the 8 NeuronCores (16 M-cells = 2 blocks of 8 per
core). Blocks carry a frozen 1-cell halo from the initial activations, so
cores never communicate.

Per core: 512 spatial blocks x 4 batches. On-chip layout: 4 tiles of 128
blocks (partition = block), free dim = padded cells x batch stored as
[i(10), j(10), k(10), b(4)] so any stencil window merges (k,b) into one
contiguous stride-1 dim of 32 -> all stencil passes hit the DVE bf16 2x
mode: window AP [(i:400,8),(j:40,8),(kb:1,32)].

Math: since normalization is affine per block, run the 27-offset weighted
sum S on RAW activations and correct afterward:
    acc = istd*(S - mu*sw + std*bias),  sw = sum_o w_o (precomputed)
so `normed` is never materialized. Stats (sum/sumsq) run on the Scalar
engine (activation accum_out), stencil products/adds on the Vector engine
in bf16, silu on ScalarE, residual update in place.

Weights are streamed from HBM each iteration (too big for SBUF),
pre-replicated x4 over batch to match the (k,b)-merged layout.
"""
import numpy as np
import ml_dtypes

BS = 8
EPS = 1e-5
NB = 16           # blocks per axis globally
N_CORES = 8
BF = ml_dtypes.bfloat16

_CACHE = {}


# ---------------------------------------------------------------- host prep

def _blk(v):
    """[..., 16,128,128] -> [..., 4(tile),128(p),512(cell)] for one core's M-slab."""
    L = v.shape[:-3]
    vr = v.reshape(*L, 2, 8, 2, 8, 8, 16, 8)   # g, ii, nh, nl, jj, kb, kk
    n = len(L)
    perm = tuple(range(n)) + tuple(n + i for i in (0, 2, 3, 5, 1, 4, 6))
    return vr.transpose(perm).reshape(*L, 4, 128, 512)


def _prep_inputs(weight, bias, residual_scale, x):
    xp = np.pad(x, ((0, 0), (1, 1), (1, 1), (1, 1)))
    swv = np.lib.stride_tricks.sliding_window_view(xp, (10, 10, 10), axis=(1, 2, 3))
    blocks = swv[:, ::8, ::8, ::8]            # [4b,16,16,16,10,10,10]
    wsum = weight.sum(axis=0)                 # [128,128,128] f32

    in_maps = []
    for c in range(N_CORES):
        g2 = blocks[:, 2 * c:2 * c + 2]       # [4b,2g,16n,16kb,10,10,10]
        arr = g2.reshape(4, 2, 2, 8, 16, 10, 10, 10)
        arr = arr.transpose(1, 2, 3, 4, 5, 6, 7, 0)   # g,nh,nl,kb,ci,cj,ck,b
        a_host = np.ascontiguousarray(arr).reshape(4, 128, 4000).astype(BF)

        wb = _blk(weight[:, 16 * c:16 * c + 16])      # [27,4,128,512]
        w4 = np.repeat(wb.transpose(1, 0, 2, 3).astype(BF)[..., None], 4, axis=-1)
        w4 = w4.reshape(4, 9, 3, 128, 2048).transpose(0, 1, 3, 2, 4)
        w4 = np.ascontiguousarray(w4).reshape(4, 9, 128, 6144)

        sw_b = np.ascontiguousarray(_blk(wsum[16 * c:16 * c + 16]))          # f32
        bias_b = np.ascontiguousarray(_blk(bias[16 * c:16 * c + 16]).astype(BF))
        rs_b = _blk(residual_scale[16 * c:16 * c + 16]).astype(BF)
        rs4 = np.ascontiguousarray(np.repeat(rs_b[..., None], 4, axis=-1)
                                   ).reshape(4, 128, 2048)
        in_maps.append({'a_in': a_host, 'w4_in': w4, 'sw_in': sw_b,
                        'bias_in': bias_b, 'rs4_in': rs4})
    return in_maps


def _unpack_outputs(results):
    y = np.empty((4, 128, 128, 128), np.float32)
    for c in range(N_CORES):
        out = results[c]['out_f']             # [4,128,2048] f32
        arr = out.reshape(2, 2, 8, 16, 8, 8, 8, 4)  # g,nh,nl,kb,ii,jj,kk,b
        yt = arr.transpose(7, 0, 4, 1, 2, 5, 3, 6)  # b,g,ii,nh,nl,jj,kb,kk
        y[:, 16 * c:16 * c + 16] = yt.reshape(4, 16, 128, 128)
    return y


# ---------------------------------------------------------------- device code

def _rng(d):
    """Incremental-window output range along one axis for offset d in {0,1,2}."""
    lo = max(0, 1 - d)
    hi = min(8, 9 - d)
    return lo, hi - lo


def _build(iters, zero_bias, unit_rs, incremental=True):
    import concourse.bass as bass
    import concourse.tile as tile
    from concourse import bacc, mybir

    BF16 = mybir.dt.bfloat16
    F32 = mybir.dt.float32
    ALU = mybir.AluOpType
    ACTF = mybir.ActivationFunctionType

    def wap(t, offset, dims):
        ap = t.ap().copy()
        ap.ap = mybir.VecI64Pair([list(t.ap().ap[0])] + [list(d) for d in dims])
        ap.offset = offset
        return ap

    nc = bacc.Bacc('TRN2', target_bir_lowering=False, debug=False)
    a_in = nc.dram_tensor('a_in', [4, 128, 4000], BF16, kind='ExternalInput').ap()
    w4_in = nc.dram_tensor('w4_in', [4, 9, 128, 6144], BF16, kind='ExternalInput').ap()
    sw_in = nc.dram_tensor('sw_in', [4, 128, 512], F32, kind='ExternalInput').ap()
    bias_in = nc.dram_tensor('bias_in', [4, 128, 512], BF16, kind='ExternalInput').ap()
    rs4_in = nc.dram_tensor('rs4_in', [4, 128, 2048], BF16, kind='ExternalInput').ap()
    out_f = nc.dram_tensor('out_f', [4, 128, 2048], F32, kind='ExternalOutput').ap()

    INTER = [[400, 8], [40, 8], [1, 32]]      # interior window dims

    with tile.TileContext(nc) as tc:
        with (
            tc.tile_pool(name='persist', bufs=1) as ppool,
            tc.tile_pool(name='wstream', bufs=4) as wpool,
            tc.tile_pool(name='work', bufs=2) as kpool,
            tc.tile_pool(name='small', bufs=2) as spool,
        ):
            A, S, D = [], [], []
            for t in range(4):
                at = ppool.tile([128, 4000], BF16, tag=f'A{t}')
                nc.sync.dma_start(at[:], a_in[t])
                A.append(at)
                if incremental:
                    st = ppool.tile([128, 2048], BF16, tag=f'S{t}')
                    S.append(st)
                    dt = ppool.tile([128, 2048], BF16, tag=f'D{t}')
                    D.append(dt)
            swt, biast, rst = [], [], []
            for t in range(4):
                s = ppool.tile([128, 512], F32, tag=f'sw{t}')
                nc.sync.dma_start(s[:], sw_in[t])
                swt.append(s)
                if not zero_bias:
                    bb = ppool.tile([128, 512], BF16, tag=f'bias{t}')
                    nc.sync.dma_start(bb[:], bias_in[t])
                    biast.append(bb)
                if not unit_rs:
                    r = ppool.tile([128, 2048], BF16, tag=f'rs{t}')
                    nc.sync.dma_start(r[:], rs4_in[t])
                    rst.append(r)

            for it in range(iters):
                last = it == iters - 1
                for t in range(4):
                    At = A[t]
                    # ---- stats on ScalarE: per-batch sum & sumsq over 1000 cells
                    s_t = spool.tile([128, 4], F32, tag='s')
                    q_t = spool.tile([128, 4], F32, tag='q')
                    junk = spool.tile([128, 1000], BF16, tag='junk')
                    junkf = spool.tile([128, 1000], F32, tag='junkf')
                    for b in range(4):
                        cells = wap(At.tensor, b, [[4, 1000]])
                        nc.scalar.activation(junk[:], cells, ACTF.Copy,
                                             accum_out=s_t[:, b:b + 1])
                        nc.scalar.activation(junkf[:], cells, ACTF.Square,
                                             accum_out=q_t[:, b:b + 1])
                    # ---- stat scalars: negmu, std, istd  [128,4]
                    negmu = spool.tile([128, 4], F32, tag='negmu')
                    nc.vector.tensor_scalar_mul(negmu[:], s_t[:], -1.0 / 1000.0)
                    m2 = spool.tile([128, 4], F32, tag='m2')
                    nc.vector.tensor_scalar_mul(m2[:], q_t[:], 1.0 / 1000.0)
                    musq = spool.tile([128, 4], F32, tag='musq')
                    nc.vector.tensor_tensor(musq[:], negmu[:], negmu[:], ALU.mult)
                    var = spool.tile([128, 4], F32, tag='var')
                    nc.vector.tensor_tensor(var[:], m2[:], musq[:], ALU.subtract)
                    nc.vector.tensor_scalar_add(var[:], var[:], EPS)
                    std = spool.tile([128, 4], F32, tag='std')
                    nc.scalar.activation(std[:], var[:], ACTF.Sqrt)
                    istd = spool.tile([128, 4], F32, tag='istd')
                    nc.vector.reciprocal(istd[:], std[:])

                    # ---- stencil (bf16, 2x mode)
                    if incremental:
                        acc = S[t]
                    else:
                        acc = kpool.tile([128, 2048], BF16, tag='acc')
                    if not incremental or it == 0:
                        # full: S = sum_o w4_o * window(A, o)
                        for c9 in range(9):
                            wt = wpool.tile([128, 6144], BF16, tag='w')
                            nc.sync.dma_start(wt[:], w4_in[t, c9])
                            for ol in range(3):
                                o = c9 * 3 + ol
                                di, dj, dk = o // 9, (o // 3) % 3, o % 3
                                win = wap(At.tensor, di * 400 + dj * 40 + dk * 4, INTER)
                                wsl = wt[:, ol * 2048:(ol + 1) * 2048]
                                if o == 0:
                                    nc.vector.tensor_tensor(acc[:], wsl, win, ALU.mult)
                                else:
                                    tmp = kpool.tile([128, 2048], BF16, tag='tmp')
                                    nc.vector.tensor_tensor(tmp[:], wsl, win, ALU.mult)
                                    nc.vector.tensor_tensor(acc[:], acc[:], tmp[:], ALU.add)
                    else:
                        # incremental: S += sum_o w4_o * window(D, o) over the
                        # sub-range where the neighbour is an interior cell
                        Dt = D[t]
                        for c9 in range(9):
                            wt = wpool.tile([128, 6144], BF16, tag='w')
                            nc.sync.dma_start(wt[:], w4_in[t, c9])
                            for ol in range(3):
                                o = c9 * 3 + ol
                                di, dj, dk = o // 9, (o // 3) % 3, o % 3
                                (li, ni), (lj, nj), (lk, nk) = _rng(di), _rng(dj), _rng(dk)
                                odims = [[256, ni], [32, nj], [1, 4 * nk]]
                                obase = li * 256 + lj * 32 + lk * 4
                                rbase = ((li + di - 1) * 256 + (lj + dj - 1) * 32
                                         + (lk + dk - 1) * 4)
                                wwin = wap(wt.tensor, ol * 2048 + obase, odims)
                                dwin = wap(Dt.tensor, rbase, odims)
                                swin = wap(acc.tensor, obase, odims)
                                nt = ni * nj * nk * 4
                                tmp = kpool.tile([128, 2048], BF16, tag='tmp')
                                tdims = [[nj * nk * 4, ni], [nk * 4, nj], [1, nk * 4]]
                                twin = wap(tmp.tensor, 0, tdims)
                                nc.vector.tensor_tensor(twin, wwin, dwin, ALU.mult)
                                nc.vector.tensor_tensor(swin, swin, twin, ALU.add)

                    # ---- correction + silu:
                    # U = S - mu*sw ; V = U + std*bias ; silu_out = Silu(istd*V)
                    U = kpool.tile([128, 2048], F32, tag='U')
                    if not zero_bias:
                        V = kpool.tile([128, 2048], F32, tag='V')
                    # silu lands in D[t] (persisted) when the next iteration
                    # needs it for the incremental stencil
                    keep_d = incremental and not last
                    if unit_rs:
                        sil = D[t] if keep_d else kpool.tile([128, 2048], BF16, tag='sil')
                    else:
                        sil = kpool.tile([128, 2048], BF16, tag='sil')
                    for b in range(4):
                        accb = wap(acc.tensor, b, [[4, 512]])
                        Ub = wap(U.tensor, b, [[4, 512]])
                        silb = wap(sil.tensor, b, [[4, 512]])
                        nc.vector.scalar_tensor_tensor(
                            Ub, swt[t][:], negmu[:, b:b + 1], accb, ALU.mult, ALU.add)
                        if not zero_bias:
                            Vb = wap(V.tensor, b, [[4, 512]])
                            nc.vector.scalar_tensor_tensor(
                                Vb, biast[t][:], std[:, b:b + 1], Ub, ALU.mult, ALU.add)
                            nc.scalar.activation(silb, Vb, ACTF.Silu,
                                                 scale=istd[:, b:b + 1])
                        else:
                            nc.scalar.activation(silb, Ub, ACTF.Silu,
                                                 scale=istd[:, b:b + 1])
                    # ---- residual update
                    if unit_rs:
                        delta = sil
                    else:
                        delta = D[t] if keep_d else kpool.tile([128, 2048], BF16, tag='delta')
                        nc.vector.tensor_tensor(delta[:], rst[t][:], sil[:], ALU.mult)
                    awin = wap(At.tensor, 444, INTER)
                    if not last:
                        nc.vector.tensor_tensor(awin, awin, delta[:], ALU.add)
                    else:
                        fo = kpool.tile([128, 2048], F32, tag='fo')
                        nc.vector.tensor_tensor(fo[:], awin, delta[:], ALU.add)
                        nc.sync.dma_start(out_f[t], fo[:])
    nc.compile()
    return nc


# ---------------------------------------------------------------- entry point

def _run_hw(weight, bias, residual_scale, x, iters):
    from concourse.bass_utils import run_bass_kernel_spmd
    zero_bias = not np.any(bias)
    unit_rs = bool(np.all(residual_scale == 1.0))
    key = (iters, zero_bias, unit_rs)
    if key not in _CACHE:
        _CACHE[key] = _build(iters, zero_bias, unit_rs)
    nc = _CACHE[key]
    in_maps = _prep_inputs(weight, bias, residual_scale, x)
    res = run_bass_kernel_spmd(nc, in_maps, core_ids=list(range(N_CORES)))
    return _unpack_outputs(res.results)


def _run_numpy(weight, bias, residual_scale, x, it):
    # reference fallback (pure numpy), used only if the HW path fails
    w = _np_blockify_param(weight)
    b = _np_blockify_param(bias)
    rs = _np_blockify_param(residual_scale)
    xp = np.pad(x, ((0, 0), (1, 1), (1, 1), (1, 1)))
    sw = np.lib.stride_tricks.sliding_window_view(xp, (10, 10, 10), axis=(1, 2, 3))
    acts = np.ascontiguousarray(sw[:, ::8, ::8, ::8])
    inter = slice(1, 9)
    for _ in range(it):
        mu = acts.mean(axis=(-3, -2, -1), keepdims=True)
        var = acts.var(axis=(-3, -2, -1), keepdims=True)
        normed = (acts - mu) / np.sqrt(var + EPS)
        acc = np.broadcast_to(b[None], (4,) + b.shape).copy()
        o = 0
        for di in range(3):
            for dj in range(3):
                for dk in range(3):
                    acc += w[o][None] * normed[..., di:di + 8, dj:dj + 8, dk:dk + 8]
                    o += 1
        acts[..., inter, inter, inter] += rs[None] * (acc / (1.0 + np.exp(-acc)))
    out = acts[..., inter, inter, inter]
    return out.transpose(0, 1, 4, 2, 5, 3, 6).reshape(4, 128, 128, 128).astype(np.float32)


def _np_blockify_param(p):
    lead = p.shape[:-3]
    y = p.reshape(*lead, 16, 8, 16, 8, 16, 8)
    n = len(lead)
    perm = tuple(range(n)) + (n, n + 2, n + 4, n + 1, n + 3, n + 5)
    return np.transpose(y, perm)


def kernel(weight, bias, residual_scale, x, inner_iterations, block_size):
    weight = np.asarray(weight, np.float32)
    bias = np.asarray(bias, np.float32)
    residual_scale = np.asarray(residual_scale, np.float32)
    x = np.asarray(x, np.float32)
    it = int(inner_iterations)
    assert int(block_size) == BS and x.shape == (4, 128, 128, 128)
    try:
        return _run_hw(weight, bias, residual_scale, x, it)
    except Exception:
        import traceback
        traceback.print_exc()
        return _run_numpy(weight, bias, residual_scale, x, it)
